# revision 13
# baseline (speedup 1.0000x reference)
"""BitLinear (bit-decoded weights + STE quant) Trainium2 kernel, v7.

y = x @ W^T + b, where
  W = decode_bits(bweight, wsign) * scale,  b = decode_bits(bbias, bsign)
      * biasscale; STE forward == identity on the already-quantized decode.

Decode: n = sum_k bits[..., k] * 2^(7-k) (exact integers 0..255),
        W = n * sign(wsign) * (scale/255).

Device strategy per core (2 token-groups x 4 out-feature-groups grid):
  - bit planes ship as fp8 with per-plane constant encoding {0, 2^(7-k)}
    (a per-element LUT, exact in fp8e4m3). The decode REDUCTION runs in
    the DMA engines' CCE (collective-compute) units: per iteration, 8
    SWDGE accumulate-DMAs sum planes 0-3 into an fp8 "hi" accumulator
    (partial sums <= 240, all fp8e4m3-exact) and planes 4-7 into "lo"
    (<= 15, exact), sliced at the CCE 2048-element descriptor cap.
    The PE never touches decode - its only work is the main matmul.
  - W_int assembly on DVE: W = (hi + lo) (exact fp16 integers <= 255),
    then W *= sign(wsign); sign is computed on DVE via clamp(ws * 1e38)
    so the ACT engine stays dedicated to the y path. All of this overlaps
    the PREVIOUS iteration's main matmuls (W double-buffered), so the PE
    runs main matmuls back-to-back across iterations.
  - main matmul: psum[o=128, t=512] += W_int^T[i,o-blk] @ x^T[i,t-chunk]
    in fp16, fp32 PSUM accumulation; t-group-major with triple-buffered
    x streaming, all 8 PSUM banks.
  - eviction on ACT: y^T = Identity(psum * (scale/255) + bias_o), bf16
    output; y DMA on the ACT HWDGE ring. x and wsign stream on the SP
    HWDGE ring; the CCE accumulates run on the Pool/SWDGE queue, so the
    three streams never head-of-line block each other.

Distribution over 8 NeuronCores: no collectives - each core writes its
own y^T shard, host reassembles.

Host-side work is layout/precision only: transposes, shard slicing, and
dtype conversion (bit planes -> fp8 with per-plane encoding, x -> fp16,
wsign -> bf16). All cross-element arithmetic (the bit-plane summation,
sign, scaling, matmul, bias) runs on the device.
"""

import numpy as np

import concourse.mybir as mybir
import concourse.tile as tile
from concourse import bacc
from concourse import bass_utils

# ---- problem constants (hardcoded per contract) ----
B, S, IN, OUT, NB = 4, 2048, 2048, 2048, 8
T = B * S                      # 8192 tokens
P = 128                        # partitions
P_T, P_O = 2, 4                # token-parallel x out-feature-parallel grid
N_CORES = P_T * P_O
T_SH = T // P_T                # 4096
O_SH = OUT // P_O              # 512
KB = IN // P                   # 16 contraction blocks
OB = O_SH // P                 # 4 out blocks
TGW = 512                      # t-group width
NT = T_SH // TGW               # 8 t-groups per core
NW = KB * O_SH                 # 8192 weight elems per partition

F32 = mybir.dt.float32
FP16 = mybir.dt.float16
BF16 = mybir.dt.bfloat16
FP8 = mybir.dt.float8e4
AL = mybir.AluOpType
IDENT = mybir.ActivationFunctionType.Identity

_CACHE = {}

import os as _os
_NO_DECODE = _os.environ.get("K_NO_DECODE", "0") == "1"  # memset W instead
_NO_MAIN = _os.environ.get("K_NO_MAIN", "0") == "1"      # decode only


def _pairs(ap):
    """Split the last (fast) axis of a [..., 2n] AP into even/odd views."""
    v = ap.rearrange("p (c two) -> p c two", two=2)
    return v[:, :, 0], v[:, :, 1]


def _build_nc(repeats=1):
    nc = bacc.Bacc("TRN2", target_bir_lowering=False, debug=False,
                   num_devices=N_CORES)

    xgd = nc.dram_tensor("xg", [NT * P, KB * TGW], FP16,
                         kind="ExternalInput").ap()
    # pre-weighted bit planes, plane-major, each plane in W layout
    bits = nc.dram_tensor("bits", [P, NB * NW], FP8,
                          kind="ExternalInput").ap()
    wsd = nc.dram_tensor("ws", [P, NW], BF16, kind="ExternalInput").ap()
    bb = nc.dram_tensor("bb", [P, OB * NB], F32, kind="ExternalInput").ap()
    bs = nc.dram_tensor("bs", [P, OB], F32, kind="ExternalInput").ap()
    scl = nc.dram_tensor("scl", [P, 1], F32, kind="ExternalInput").ap()
    bscl = nc.dram_tensor("bscl", [P, 1], F32, kind="ExternalInput").ap()
    y = nc.dram_tensor("y", [O_SH, T_SH], BF16, kind="ExternalOutput").ap()

    bits3 = bits.rearrange("p (k w) -> p k w", k=NB)

    with tile.TileContext(nc) as tc:
      with tc.tile_pool(name="w", bufs=1) as wpool, \
           tc.tile_pool(name="xs", bufs=1) as xpool, \
           tc.tile_pool(name="dec", bufs=1) as dec, \
           tc.tile_pool(name="yb", bufs=1) as ypool, \
           tc.tile_pool(name="psum", bufs=1, space="PSUM") as psum_pool:

        for rep in range(repeats):
            par = rep % 2   # parity suffix: double-buffer rep-crossing state

            # ---- scalars (SP ring first: short critical path to bias) ----
            scl_sb = dec.tile([P, 1], F32, name=f"scl_sb{par}")
            nc.sync.dma_start(out=scl_sb, in_=scl)
            bscl_sb = dec.tile([P, 1], F32, name=f"bscl_sb{par}")
            nc.sync.dma_start(out=bscl_sb, in_=bscl)
            bb_sb = dec.tile([P, OB * NB], F32, name=f"bb_sb{par}")
            nc.sync.dma_start(out=bb_sb, in_=bb)
            bs_sb = dec.tile([P, OB], F32, name=f"bs_sb{par}")
            nc.sync.dma_start(out=bs_sb, in_=bs)

            # ---- weight sign source (bf16, sign formed on DVE below) ----
            wsg = dec.tile([P, NW], BF16, name=f"wsg{par}")
            nc.sync.dma_start(out=wsg, in_=wsd)

            # ---- first 3 x tiles next on the SP ring ----
            def load_xg(g):
                xt = xpool.tile([P, KB * TGW], FP16, tag="xg",
                                name=f"xg{g}_{rep}", bufs=4)
                nc.sync.dma_start(out=xt, in_=xgd[g * P:(g + 1) * P, :])
                return xt

            xtiles = {g: load_xg(g) for g in range(min(4, NT))}

            # ---- bit-plane decode in the DMA engines (CCE accumulate):
            #      hi = sum planes 0-3, lo = sum planes 4-7 (both exact in
            #      fp8); sliced at the CCE 2048-elem descriptor cap ----
            hi = dec.tile([P, NW], FP8, name=f"hi{par}")
            lo = dec.tile([P, NW], FP8, name=f"lo{par}")
            SW = 2048

            # ---- scalar prep + bias decode (DVE/ACT) ----
            s255 = dec.tile([P, 1], F32, name=f"s255_{par}")
            nc.vector.tensor_scalar_mul(s255, scl_sb, 1.0 / 255.0)
            bs255 = dec.tile([P, 1], F32, name=f"bs255_{par}")
            nc.vector.tensor_scalar_mul(bs255, bscl_sb, 1.0 / 255.0)
            e, o = _pairs(bb_sb)
            bl1 = dec.tile([P, OB * 4], F32, name=f"bl1_{par}")
            nc.vector.scalar_tensor_tensor(out=bl1, in0=e, scalar=2.0,
                                           in1=o, op0=AL.mult, op1=AL.add)
            e, o = _pairs(bl1)
            bl2 = dec.tile([P, OB * 2], F32, name=f"bl2_{par}")
            nc.vector.scalar_tensor_tensor(out=bl2, in0=e, scalar=4.0,
                                           in1=o, op0=AL.mult, op1=AL.add)
            e, o = _pairs(bl2)
            bl3 = dec.tile([P, OB], F32, name=f"bl3_{par}")
            nc.vector.scalar_tensor_tensor(out=bl3, in0=e, scalar=16.0,
                                           in1=o, op0=AL.mult, op1=AL.add)
            bsg = dec.tile([P, OB], F32, name=f"bsg{par}")
            nc.scalar.sign(bsg, bs_sb)
            bias_col = dec.tile([P, OB], F32, name=f"bias_col{par}")
            nc.vector.scalar_tensor_tensor(out=bias_col, in0=bl3,
                                           scalar=bs255, in1=bsg,
                                           op0=AL.mult, op1=AL.mult)

            # ---- sign(wsign) on DVE, in place: clamp(wsg * 1e38, -1, 1).
            # bf16-normal |wsg| >= 1.2e-38 -> *1e38 >= 1.17 -> clamps to
            # +-1 exactly; overflow saturates to +-inf then clamps. ----
            nc.vector.tensor_scalar(out=wsg, in0=wsg, scalar1=1e38,
                                    scalar2=1.0, op0=AL.mult, op1=AL.min)
            nc.vector.tensor_scalar(out=wsg, in0=wsg, scalar1=-1.0,
                                    scalar2=None, op0=AL.max)

            # ---- decode in the DMA engines (CCE accumulate) + W_int
            #      assembly on DVE, pipelined in kb-halves so the first
            #      main matmuls only wait for half the chain ----
            W = wpool.tile([P, NW], FP16, name=f"W{par}")
            if _NO_DECODE:
                nc.vector.memset(W, 1.0)
                nc.vector.tensor_tensor(out=W, in0=W, in1=wsg, op=AL.mult)
            else:
                for h0 in range(0, NW, SW):
                    hsl = slice(h0, h0 + SW)
                    for k in range(4):
                        nc.gpsimd.dma_start(
                            out=hi[:, hsl], in_=bits3[:, k, hsl],
                            accum_op=(AL.bypass if k == 0 else AL.add))
                    for k in range(4, 8):
                        nc.gpsimd.dma_start(
                            out=lo[:, hsl], in_=bits3[:, k, hsl],
                            accum_op=(AL.bypass if k == 4 else AL.add))
                    # W_quarter = (hi + lo) * sign  (exact fp16 ints)
                    nc.vector.scalar_tensor_tensor(
                        out=W[:, hsl], in0=hi[:, hsl], scalar=1.0,
                        in1=lo[:, hsl], op0=AL.mult, op1=AL.add)
                    nc.vector.tensor_tensor(out=W[:, hsl], in0=W[:, hsl],
                                            in1=wsg[:, hsl], op=AL.mult)
            W3 = W.rearrange("p (kb o) -> p kb o", kb=KB)

            # ---- main matmul: t-group-major, x triple-buffered ----
            for g in range(NT if not _NO_MAIN else 0):
                xg3 = xtiles[g].rearrange("p (kb t) -> p kb t", kb=KB)
                ybuf = ypool.tile([P, OB * TGW], BF16, tag="yb", bufs=3)
                yb3 = ybuf.rearrange("p (ob t) -> p ob t", ob=OB)
                for ob in range(OB):
                    ps = psum_pool.tile([P, TGW], F32, tag="mm", bufs=8)
                    for kb in range(KB):
                        nc.tensor.matmul(
                            ps,
                            W3[:, kb, ob * P:(ob + 1) * P],
                            xg3[:, kb],
                            start=(kb == 0),
                            stop=(kb == KB - 1),
                        )
                    # y^T tile = psum * (scale/255) + bias_o   (ACT)
                    nc.scalar.activation(
                        out=yb3[:, ob], in_=ps, func=IDENT,
                        bias=bias_col[:, ob:ob + 1], scale=s255)
                if g + 4 < NT:
                    xtiles[g + 4] = load_xg(g + 4)
                # y on the ACT ring so it never head-of-line blocks the
                # SP-ring x prefetch stream
                nc.scalar.dma_start(
                    out=y.rearrange("(ob p) t -> p ob t", p=P)[
                        :, :, g * TGW:(g + 1) * TGW],
                    in_=yb3,
                )

    nc.compile()
    return nc


def _shard_inputs(x, bweight, wsign, scale, bbias, bsign, biasscale):
    fp8_np = mybir.dt.np(FP8)
    bf16_np = mybir.dt.np(BF16)

    x2 = np.asarray(x, dtype=np.float32).reshape(T, IN)
    bwf = np.asarray(bweight, dtype=np.float32)
    wsf = np.asarray(wsign, dtype=np.float32)
    bbias = np.asarray(bbias, dtype=np.float32)
    bsign = np.asarray(bsign, dtype=np.float32)

    scl_rep = np.full((P, 1), np.asarray(scale).reshape(-1)[0],
                      dtype=np.float32)
    bscl_rep = np.full((P, 1), np.asarray(biasscale).reshape(-1)[0],
                       dtype=np.float32)

    o_maps = []
    for o_grp in range(P_O):
        osl = slice(o_grp * O_SH, (o_grp + 1) * O_SH)
        bw_sh = bwf[osl]                              # [O_SH, IN, NB]
        # plane k in W layout [p, kb*O_SH + o] = bit(i=kb*128+p, o, k),
        # encoded as {0, 2^(7-k)} (fp8-exact LUT)
        planes = []
        for k in range(NB):
            pl = (bw_sh[:, :, k].T * (2.0 ** (7 - k)))   # [IN, O_SH]
            planes.append(
                pl.reshape(KB, P, O_SH).transpose(1, 0, 2).reshape(P, NW))
        bits_np = np.ascontiguousarray(
            np.concatenate(planes, axis=1)).astype(fp8_np)
        # ws: [p, kb*O_SH + o] = sign weight for (i = kb*128+p, o)
        ws_np = np.ascontiguousarray(
            wsf[osl].T.reshape(KB, P, O_SH).transpose(1, 0, 2)
            .reshape(P, NW)).astype(bf16_np)
        o_maps.append({
            "bits": bits_np,
            "ws": ws_np,
            "bb": np.ascontiguousarray(
                bbias[osl].reshape(OB, P, NB).transpose(1, 0, 2)
                .reshape(P, OB * NB)),
            "bs": np.ascontiguousarray(bsign[osl].reshape(OB, P).T),
            "scl": scl_rep,
            "bscl": bscl_rep,
        })

    in_maps = [None] * N_CORES
    for t_grp in range(P_T):
        tsl = slice(t_grp * T_SH, (t_grp + 1) * T_SH)
        xs = x2[tsl]                                  # [T_SH, IN]
        xg_np = np.ascontiguousarray(
            xs.reshape(NT, TGW, KB, P).transpose(0, 3, 2, 1)
            .reshape(NT * P, KB * TGW).astype(np.float16))
        for o_grp in range(P_O):
            c = t_grp * P_O + o_grp
            in_maps[c] = dict(o_maps[o_grp], xg=xg_np)
    return in_maps


def kernel(x, bweight, wsign, scale, bbias, bsign, biasscale):
    if "nc" not in _CACHE:
        _CACHE["nc"] = _build_nc()
    nc = _CACHE["nc"]
    in_maps = _shard_inputs(x, bweight, wsign, scale, bbias, bsign, biasscale)
    res = bass_utils.run_bass_kernel_spmd(
        nc, in_maps, core_ids=list(range(N_CORES)))
    Y = np.empty((T, OUT), dtype=np.float32)
    for c in range(N_CORES):
        t_grp, o_grp = c // P_O, c % P_O
        Y[t_grp * T_SH:(t_grp + 1) * T_SH,
          o_grp * O_SH:(o_grp + 1) * O_SH] = \
            res.results[c]["y"].T.astype(np.float32)
    return Y.reshape(B, S, OUT)


# revision 14
# speedup vs baseline: 1.0334x; 1.0334x over previous
"""BitLinear (bit-decoded weights + STE quant) Trainium2 kernel, v7.

y = x @ W^T + b, where
  W = decode_bits(bweight, wsign) * scale,  b = decode_bits(bbias, bsign)
      * biasscale; STE forward == identity on the already-quantized decode.

Decode: n = sum_k bits[..., k] * 2^(7-k) (exact integers 0..255),
        W = n * sign(wsign) * (scale/255).

Device strategy per core (2 token-groups x 4 out-feature-groups grid):
  - bit planes ship as fp8 with per-plane constant encoding {0, 2^(7-k)}
    (a per-element LUT, exact in fp8e4m3). The decode REDUCTION runs in
    the DMA engines' CCE (collective-compute) units: per iteration, 8
    SWDGE accumulate-DMAs sum planes 0-3 into an fp8 "hi" accumulator
    (partial sums <= 240, all fp8e4m3-exact) and planes 4-7 into "lo"
    (<= 15, exact), sliced at the CCE 2048-element descriptor cap.
    The PE never touches decode - its only work is the main matmul.
  - W_int assembly on DVE: W = (hi + lo) (exact fp16 integers <= 255),
    then W *= sign(wsign); sign is computed on DVE via clamp(ws * 1e38)
    so the ACT engine stays dedicated to the y path. All of this overlaps
    the PREVIOUS iteration's main matmuls (W double-buffered), so the PE
    runs main matmuls back-to-back across iterations.
  - main matmul: psum[o=128, t=512] += W_int^T[i,o-blk] @ x^T[i,t-chunk]
    in fp16, fp32 PSUM accumulation; t-group-major with triple-buffered
    x streaming, all 8 PSUM banks.
  - eviction on ACT: y^T = Identity(psum * (scale/255) + bias_o), bf16
    output; y DMA on the ACT HWDGE ring. x and wsign stream on the SP
    HWDGE ring; the CCE accumulates run on the Pool/SWDGE queue, so the
    three streams never head-of-line block each other.

Distribution over 8 NeuronCores: no collectives - each core writes its
own y^T shard, host reassembles.

Host-side work is layout/precision only: transposes, shard slicing, and
dtype conversion (bit planes -> fp8 with per-plane encoding, x -> fp16,
wsign -> bf16). All cross-element arithmetic (the bit-plane summation,
sign, scaling, matmul, bias) runs on the device.
"""

import numpy as np

import concourse.mybir as mybir
import concourse.tile as tile
from concourse import bacc
from concourse import bass_utils

# ---- problem constants (hardcoded per contract) ----
B, S, IN, OUT, NB = 4, 2048, 2048, 2048, 8
T = B * S                      # 8192 tokens
P = 128                        # partitions
P_T, P_O = 2, 4                # token-parallel x out-feature-parallel grid
N_CORES = P_T * P_O
T_SH = T // P_T                # 4096
O_SH = OUT // P_O              # 512
KB = IN // P                   # 16 contraction blocks
OB = O_SH // P                 # 4 out blocks
TGW = 512                      # t-group width
NT = T_SH // TGW               # 8 t-groups per core
NW = KB * O_SH                 # 8192 weight elems per partition

F32 = mybir.dt.float32
FP16 = mybir.dt.float16
BF16 = mybir.dt.bfloat16
FP8 = mybir.dt.float8e4
AL = mybir.AluOpType
IDENT = mybir.ActivationFunctionType.Identity

_CACHE = {}

import os as _os
_NO_DECODE = _os.environ.get("K_NO_DECODE", "0") == "1"  # memset W instead
_NO_MAIN = _os.environ.get("K_NO_MAIN", "0") == "1"      # decode only


def _pairs(ap):
    """Split the last (fast) axis of a [..., 2n] AP into even/odd views."""
    v = ap.rearrange("p (c two) -> p c two", two=2)
    return v[:, :, 0], v[:, :, 1]


def _build_nc(repeats=1):
    nc = bacc.Bacc("TRN2", target_bir_lowering=False, debug=False,
                   num_devices=N_CORES)

    xgd = nc.dram_tensor("xg", [NT * P, KB * TGW], FP16,
                         kind="ExternalInput").ap()
    # pre-weighted bit planes, plane-major, each plane in W layout
    bits = nc.dram_tensor("bits", [P, NB * NW], FP8,
                          kind="ExternalInput").ap()
    wsd = nc.dram_tensor("ws", [P, NW], BF16, kind="ExternalInput").ap()
    bb = nc.dram_tensor("bb", [P, OB * NB], F32, kind="ExternalInput").ap()
    bs = nc.dram_tensor("bs", [P, OB], F32, kind="ExternalInput").ap()
    scl = nc.dram_tensor("scl", [P, 1], F32, kind="ExternalInput").ap()
    bscl = nc.dram_tensor("bscl", [P, 1], F32, kind="ExternalInput").ap()
    y = nc.dram_tensor("y", [O_SH, T_SH], BF16, kind="ExternalOutput").ap()

    bits3 = bits.rearrange("p (k w) -> p k w", k=NB)

    with tile.TileContext(nc) as tc:
      with tc.tile_pool(name="w", bufs=1) as wpool, \
           tc.tile_pool(name="xs", bufs=1) as xpool, \
           tc.tile_pool(name="dec", bufs=1) as dec, \
           tc.tile_pool(name="yb", bufs=1) as ypool, \
           tc.tile_pool(name="psum", bufs=1, space="PSUM") as psum_pool:

        for rep in range(repeats):
            par = rep % 2   # parity suffix: double-buffer rep-crossing state

            # ---- scalars (SP ring first: short critical path to bias) ----
            scl_sb = dec.tile([P, 1], F32, name=f"scl_sb{par}")
            nc.sync.dma_start(out=scl_sb, in_=scl)
            bscl_sb = dec.tile([P, 1], F32, name=f"bscl_sb{par}")
            nc.sync.dma_start(out=bscl_sb, in_=bscl)
            bb_sb = dec.tile([P, OB * NB], F32, name=f"bb_sb{par}")
            nc.sync.dma_start(out=bb_sb, in_=bb)
            bs_sb = dec.tile([P, OB], F32, name=f"bs_sb{par}")
            nc.sync.dma_start(out=bs_sb, in_=bs)

            # ---- weight sign source (bf16, sign formed on DVE below) ----
            wsg = dec.tile([P, NW], BF16, name=f"wsg{par}")
            nc.sync.dma_start(out=wsg, in_=wsd)

            # ---- first 3 x tiles next on the SP ring ----
            def load_xg(g):
                xt = xpool.tile([P, KB * TGW], FP16, tag="xg",
                                name=f"xg{g}_{rep}", bufs=4)
                nc.sync.dma_start(out=xt, in_=xgd[g * P:(g + 1) * P, :])
                return xt

            xtiles = {g: load_xg(g) for g in range(min(4, NT))}

            # ---- bit-plane decode in the DMA engines (CCE accumulate):
            #      hi = sum planes 0-3, lo = sum planes 4-7 (both exact in
            #      fp8); sliced at the CCE 2048-elem descriptor cap ----
            hi = dec.tile([P, NW], FP8, name=f"hi{par}")
            lo = dec.tile([P, NW], FP8, name=f"lo{par}")
            SW = 2048

            # ---- scalar prep + bias decode (DVE/ACT) ----
            s255 = dec.tile([P, 1], F32, name=f"s255_{par}")
            nc.vector.tensor_scalar_mul(s255, scl_sb, 1.0 / 255.0)
            bs255 = dec.tile([P, 1], F32, name=f"bs255_{par}")
            nc.vector.tensor_scalar_mul(bs255, bscl_sb, 1.0 / 255.0)
            e, o = _pairs(bb_sb)
            bl1 = dec.tile([P, OB * 4], F32, name=f"bl1_{par}")
            nc.vector.scalar_tensor_tensor(out=bl1, in0=e, scalar=2.0,
                                           in1=o, op0=AL.mult, op1=AL.add)
            e, o = _pairs(bl1)
            bl2 = dec.tile([P, OB * 2], F32, name=f"bl2_{par}")
            nc.vector.scalar_tensor_tensor(out=bl2, in0=e, scalar=4.0,
                                           in1=o, op0=AL.mult, op1=AL.add)
            e, o = _pairs(bl2)
            bl3 = dec.tile([P, OB], F32, name=f"bl3_{par}")
            nc.vector.scalar_tensor_tensor(out=bl3, in0=e, scalar=16.0,
                                           in1=o, op0=AL.mult, op1=AL.add)
            bsg = dec.tile([P, OB], F32, name=f"bsg{par}")
            nc.scalar.sign(bsg, bs_sb)
            bias_col = dec.tile([P, OB], F32, name=f"bias_col{par}")
            nc.vector.scalar_tensor_tensor(out=bias_col, in0=bl3,
                                           scalar=bs255, in1=bsg,
                                           op0=AL.mult, op1=AL.mult)

            # ---- sign(wsign) on DVE, in place: clamp(wsg * 1e38, -1, 1).
            # bf16-normal |wsg| >= 1.2e-38 -> *1e38 >= 1.17 -> clamps to
            # +-1 exactly; overflow saturates to +-inf then clamps. ----
            nc.vector.tensor_scalar(out=wsg, in0=wsg, scalar1=1e38,
                                    scalar2=1.0, op0=AL.mult, op1=AL.min)
            nc.vector.tensor_scalar(out=wsg, in0=wsg, scalar1=-1.0,
                                    scalar2=None, op0=AL.max)

            # ---- decode in the DMA engines (CCE accumulate) + W_int
            #      assembly on DVE, pipelined in kb-halves so the first
            #      main matmuls only wait for half the chain ----
            W = wpool.tile([P, NW], FP16, name=f"W{par}")
            if _NO_DECODE:
                nc.vector.memset(W, 1.0)
                nc.vector.tensor_tensor(out=W, in0=W, in1=wsg, op=AL.mult)
            else:
                # round-robin the 8 independent accumulate chains (4
                # quarters x hi/lo) so no link's completion-wait ever
                # head-of-line blocks the SWDGE queue
                for k in range(4):
                    for h0 in range(0, NW, SW):
                        hsl = slice(h0, h0 + SW)
                        nc.gpsimd.dma_start(
                            out=hi[:, hsl], in_=bits3[:, k, hsl],
                            accum_op=(AL.bypass if k == 0 else AL.add))
                        nc.gpsimd.dma_start(
                            out=lo[:, hsl], in_=bits3[:, k + 4, hsl],
                            accum_op=(AL.bypass if k == 0 else AL.add))
                for h0 in range(0, NW, SW):
                    hsl = slice(h0, h0 + SW)
                    # W_quarter = (hi + lo) * sign  (exact fp16 ints)
                    nc.vector.scalar_tensor_tensor(
                        out=W[:, hsl], in0=hi[:, hsl], scalar=1.0,
                        in1=lo[:, hsl], op0=AL.mult, op1=AL.add)
                    nc.vector.tensor_tensor(out=W[:, hsl], in0=W[:, hsl],
                                            in1=wsg[:, hsl], op=AL.mult)
            W3 = W.rearrange("p (kb o) -> p kb o", kb=KB)

            # ---- main matmul: t-group-major, x triple-buffered ----
            for g in range(NT if not _NO_MAIN else 0):
                xg3 = xtiles[g].rearrange("p (kb t) -> p kb t", kb=KB)
                ybuf = ypool.tile([P, OB * TGW], BF16, tag="yb", bufs=3)
                yb3 = ybuf.rearrange("p (ob t) -> p ob t", ob=OB)
                for ob in range(OB):
                    ps = psum_pool.tile([P, TGW], F32, tag="mm", bufs=8)
                    for kb in range(KB):
                        nc.tensor.matmul(
                            ps,
                            W3[:, kb, ob * P:(ob + 1) * P],
                            xg3[:, kb],
                            start=(kb == 0),
                            stop=(kb == KB - 1),
                        )
                    # y^T tile = psum * (scale/255) + bias_o   (ACT)
                    nc.scalar.activation(
                        out=yb3[:, ob], in_=ps, func=IDENT,
                        bias=bias_col[:, ob:ob + 1], scale=s255)
                if g + 4 < NT:
                    xtiles[g + 4] = load_xg(g + 4)
                # y on the ACT ring so it never head-of-line blocks the
                # SP-ring x prefetch stream
                nc.scalar.dma_start(
                    out=y.rearrange("(ob p) t -> p ob t", p=P)[
                        :, :, g * TGW:(g + 1) * TGW],
                    in_=yb3,
                )

    nc.compile()
    return nc


def _shard_inputs(x, bweight, wsign, scale, bbias, bsign, biasscale):
    fp8_np = mybir.dt.np(FP8)
    bf16_np = mybir.dt.np(BF16)

    x2 = np.asarray(x, dtype=np.float32).reshape(T, IN)
    bwf = np.asarray(bweight, dtype=np.float32)
    wsf = np.asarray(wsign, dtype=np.float32)
    bbias = np.asarray(bbias, dtype=np.float32)
    bsign = np.asarray(bsign, dtype=np.float32)

    scl_rep = np.full((P, 1), np.asarray(scale).reshape(-1)[0],
                      dtype=np.float32)
    bscl_rep = np.full((P, 1), np.asarray(biasscale).reshape(-1)[0],
                       dtype=np.float32)

    o_maps = []
    for o_grp in range(P_O):
        osl = slice(o_grp * O_SH, (o_grp + 1) * O_SH)
        bw_sh = bwf[osl]                              # [O_SH, IN, NB]
        # plane k in W layout [p, kb*O_SH + o] = bit(i=kb*128+p, o, k),
        # encoded as {0, 2^(7-k)} (fp8-exact LUT)
        planes = []
        for k in range(NB):
            pl = (bw_sh[:, :, k].T * (2.0 ** (7 - k)))   # [IN, O_SH]
            planes.append(
                pl.reshape(KB, P, O_SH).transpose(1, 0, 2).reshape(P, NW))
        bits_np = np.ascontiguousarray(
            np.concatenate(planes, axis=1)).astype(fp8_np)
        # ws: [p, kb*O_SH + o] = sign weight for (i = kb*128+p, o)
        ws_np = np.ascontiguousarray(
            wsf[osl].T.reshape(KB, P, O_SH).transpose(1, 0, 2)
            .reshape(P, NW)).astype(bf16_np)
        o_maps.append({
            "bits": bits_np,
            "ws": ws_np,
            "bb": np.ascontiguousarray(
                bbias[osl].reshape(OB, P, NB).transpose(1, 0, 2)
                .reshape(P, OB * NB)),
            "bs": np.ascontiguousarray(bsign[osl].reshape(OB, P).T),
            "scl": scl_rep,
            "bscl": bscl_rep,
        })

    in_maps = [None] * N_CORES
    for t_grp in range(P_T):
        tsl = slice(t_grp * T_SH, (t_grp + 1) * T_SH)
        xs = x2[tsl]                                  # [T_SH, IN]
        xg_np = np.ascontiguousarray(
            xs.reshape(NT, TGW, KB, P).transpose(0, 3, 2, 1)
            .reshape(NT * P, KB * TGW).astype(np.float16))
        for o_grp in range(P_O):
            c = t_grp * P_O + o_grp
            in_maps[c] = dict(o_maps[o_grp], xg=xg_np)
    return in_maps


def kernel(x, bweight, wsign, scale, bbias, bsign, biasscale):
    if "nc" not in _CACHE:
        _CACHE["nc"] = _build_nc()
    nc = _CACHE["nc"]
    in_maps = _shard_inputs(x, bweight, wsign, scale, bbias, bsign, biasscale)
    res = bass_utils.run_bass_kernel_spmd(
        nc, in_maps, core_ids=list(range(N_CORES)))
    Y = np.empty((T, OUT), dtype=np.float32)
    for c in range(N_CORES):
        t_grp, o_grp = c // P_O, c % P_O
        Y[t_grp * T_SH:(t_grp + 1) * T_SH,
          o_grp * O_SH:(o_grp + 1) * O_SH] = \
            res.results[c]["y"].T.astype(np.float32)
    return Y.reshape(B, S, OUT)
